# revision 33
# baseline (speedup 1.0000x reference)
"""Trainium2 Bass kernel for ComputeGsct.

Math (per batch b, reduced over N voxels):
    kai(n)   = 10*x2[n,0] - i * x2[n,1]/(OMEGA*EPS0)          (complex scalar)
    A_n      = kai(n) * Gsr_n                                  (complex 3x3)
    C_b      = sum_n A_n @ Grf_n                               (complex 3x3)
    out[b,m,:] = (Re C_b, Im C_b) flattened row-major.

Strategy (v8):
  - Batch-parallel sharding: 8 cores x 4 batches each, full N per core.
    Output is concatenated on host - no cross-core reduction needed.
  - Host converts x0/x1 to f16 before upload.  The on-chip pipeline
    rounds Gsr/Grf to f16 before the matmuls anyway (PE runs f16), so
    this halves HBM traffic (79.7 -> 41.9 MB/core) at identical
    numerics class; rel err stays ~4e-4 vs the 2e-2 gate.
  - The complex combine lives OFF-DEVICE: per 128-voxel chunk the PE
    accumulates TWO [18,18]-blocked products into separate PSUM tiles,
      ps_r += (kr*Gsr)^T Grf      ps_i += (ki*Gsr)^T Grf
    (diag-block trick, KGRP chunks per matmul).  All eight real-product
    sums needed for the complex 3x3 result are linear combinations of
    ps_r/ps_i entries - a tiny host-side fixup extracts them.
  - On-chip per tile: two tiny ACT muls (kr, ki -> f16) and two f16
    DVE broadcast-muls (kr*Gsr, ki*Gsr).  Grf feeds the PE directly.
  - DMA: x0 on the sync(SP) ring, x1 on the scalar(ACT) ring - whole-
    tensor transfers only (splitting one tensor across rings interleaves
    the queues' packets on the shared DMA engines and HALVES per-engine
    throughput - measured).  xk rides the otherwise-idle Pool
    (gpsimd software-DGE) queue so the big rings are pure streamers.
"""

import sys

import numpy as np

_TRN_REPO = "/opt/trn_rl_repo"
if _TRN_REPO not in sys.path:
    sys.path.insert(0, _TRN_REPO)

_PAI = 3.141592653589793
_C = 299792458.0
_OMEGA = 2.0 * _PAI * 2.4e9
_MU0 = 4.0 * _PAI * 1e-7
_EPSILON0 = 1.0 / (_C**2 * _MU0)
_KI_SCALE = -1.0 / (_OMEGA * _EPSILON0)

B_FULL, N_FULL = 32, 131072
N_CORES = 8
B_PC = B_FULL // N_CORES  # batches per core
P = 128  # SBUF partitions == matmul contraction size
KGRP = 4  # voxel-chunks fused per matmul (diag-block trick)
SW = 18  # stationary cols per chunk: Grf
MW = 36  # moving cols per chunk: [kr*Gsr | ki*Gsr]
FD = KGRP * SW  # psum partition dim (72)
FM = KGRP * MW  # psum free dim (144)


def build_nc(b_pc=B_PC, n=N_FULL, q=128, repeat=1):
    """Build the per-core Bass program (SPMD: same program, per-core data)."""
    from contextlib import ExitStack

    import concourse.bacc as bacc
    import concourse.mybir as mybir
    from concourse import tile
    from concourse.bass import ts

    f32 = mybir.dt.float32
    f16 = mybir.dt.float16
    nc = bacc.Bacc("TRN2", target_bir_lowering=False, debug=False)

    x0 = nc.dram_tensor("x0", [b_pc, n, 9, 2], f16, kind="ExternalInput")
    x1 = nc.dram_tensor("x1", [b_pc, n, 9, 2], f16, kind="ExternalInput")
    x2 = nc.dram_tensor("x2", [b_pc, n, 2], f16, kind="ExternalInput")
    out = nc.dram_tensor("out", [FD, b_pc * 2 * FM], f32, kind="ExternalOutput")

    tile_v = P * q  # voxels per tile iteration
    assert n % tile_v == 0
    n_tiles = n // tile_v

    with ExitStack() as ctx:
        tc = ctx.enter_context(tile.TileContext(nc))
        io_g0 = ctx.enter_context(tc.tile_pool(name="io_g0", bufs=5))
        io_g1 = ctx.enter_context(tc.tile_pool(name="io_g1", bufs=5))
        io_xk = ctx.enter_context(tc.tile_pool(name="io_xk", bufs=6))
        work = ctx.enter_context(tc.tile_pool(name="work", bufs=6))
        psum = ctx.enter_context(tc.tile_pool(name="psum", bufs=2, space="PSUM"))
        outp = ctx.enter_context(tc.tile_pool(name="outp", bufs=1))

        if repeat > 1:
            loop = ctx.enter_context(tc.For_i(0, repeat, 1))  # noqa: F841

        stage = outp.tile([FD, b_pc * 2 * FM], f32)

        DT = 2  # compute-tiles per big-ring DMA (doubles descriptor size)
        assert n_tiles % DT == 0

        for b in range(b_pc):
            psA = psum.tile([FD, FM], f32, tag="psA")
            psB = psum.tile([FD, FM], f32, tag="psB")
            g0d = g1d = xkd = None
            for t in range(n_tiles):
                # ---- loads: voxel v = t*tile_v + p*q + qq, contiguous per
                # partition.  g0/g1 load DT tiles per transfer so each
                # descriptor is DT*q*18*2 bytes (bigger descriptors lift
                # the per-ring rate); xk stays per-tile on the Pool queue.
                if t % DT == 0:
                    td = t // DT
                    xkd = io_xk.tile([P, DT * q * 2], f16, tag="xk")
                    nc.gpsimd.dma_start(
                        xkd[:],
                        x2[b, ts(td, DT * tile_v)].rearrange(
                            "(p qq) r -> p (qq r)", p=P
                        ),
                    )
                    g0s = x0[b, ts(td, DT * tile_v)].rearrange(
                        "(p qq) m r -> p (qq m r)", p=P
                    )
                    g1s = x1[b, ts(td, DT * tile_v)].rearrange(
                        "(p qq) m r -> p (qq m r)", p=P
                    )
                    g0d = io_g0.tile([P, DT * q * 18], f16, tag="g0")
                    g1d = io_g1.tile([P, DT * q * 18], f16, tag="g1")
                    ends = (b == 0 and td == 0) or (
                        b == b_pc - 1 and td == n_tiles // DT - 1
                    )
                    if ends:
                        # halve the first/last transfers: the pipeline
                        # primes (first compute waits only ~1.5us) and
                        # drains (last compute starts earlier) faster.
                        HW_ = q * 18
                        nc.sync.dma_start(g0d[:, 0:HW_], g0s[:, 0:HW_])
                        nc.sync.dma_start(g0d[:, HW_:], g0s[:, HW_:])
                        nc.scalar.dma_start(g1d[:, 0:HW_], g1s[:, 0:HW_])
                        nc.scalar.dma_start(g1d[:, HW_:], g1s[:, HW_:])
                    elif td % 4 == 3:
                        # every 4th double-transfer rides the Pool queue:
                        # three balanced streams (~13.4MB each) instead of
                        # two 19MB rings + idle Pool
                        nc.gpsimd.dma_start(g0d[:], g0s)
                        nc.gpsimd.dma_start(g1d[:], g1s)
                    else:
                        nc.sync.dma_start(g0d[:], g0s)
                        nc.scalar.dma_start(g1d[:], g1s)
                h = t % DT
                g0 = g0d[:, h * q * 18 : (h + 1) * q * 18]
                g1 = g1d[:, h * q * 18 : (h + 1) * q * 18]
                xk = xkd[:, h * q * 2 : (h + 1) * q * 2]

                # ---- kai components in f16, duplicated pairwise (ACT):
                # krd[p, 2*v] = krd[p, 2*v+1] = 10*x2[v,0].  The pair
                # duplication lets the DVE muls read kai through an AP
                # whose LAST dim is stride-1 x 2 elements — the packed-
                # last-dim requirement of DVE's 2x_1p fast path (a plain
                # stride-0 broadcast runs at 1 elem/cycle/lane).
                xkv = xk.rearrange("p (qq r) -> p qq r", r=2)
                krd = work.tile([P, q * 2], f16, tag="krd")
                krdv = krd[:].rearrange("p (qq d) -> p qq d", d=2)
                nc.vector.tensor_scalar_mul(krdv[:, :, 0], xkv[:, :, 0], 10.0)
                nc.vector.tensor_scalar_mul(krdv[:, :, 1], xkv[:, :, 0], 10.0)
                kid = work.tile([P, q * 2], f16, tag="kid")
                kidv = kid[:].rearrange("p (qq d) -> p qq d", d=2)
                nc.vector.tensor_scalar_mul(kidv[:, :, 0], xkv[:, :, 1], _KI_SCALE)
                nc.vector.tensor_scalar_mul(kidv[:, :, 1], xkv[:, :, 1], _KI_SCALE)

                # ---- scaled stationary (DVE, f16 2x path): interleaved
                # [kr*Gsr | ki*Gsr] per voxel chunk, 36 cols each.
                ss = work.tile([P, q * 36], f16, tag="ss")
                ss5 = ss[:].rearrange("p (qq h m r) -> p qq h m r", h=2, m=9, r=2)
                g0v4 = g0.rearrange("p (qq m r) -> p qq m r", m=9, r=2)
                nc.vector.tensor_mul(
                    ss5[:, :, 0],
                    g0v4,
                    krdv[:].unsqueeze(2).broadcast_to((P, q, 9, 2)),
                )
                nc.vector.tensor_mul(
                    ss5[:, :, 1],
                    g0v4,
                    kidv[:].unsqueeze(2).broadcast_to((P, q, 9, 2)),
                )

                # ---- TensorE: per 4-chunk group, ONE [128,72]^T@[128,144]
                # matmul (stationary = Grf, moving = interleaved scaled
                # Gsr); the 4 diagonal [18,36] blocks hold the wanted
                # kr/ki-scaled products.  PE is sequencer-dispatch-bound
                # (~30ns/inst), so fewer+fatter matmuls win.
                ssv = ss[:].rearrange("p (g c) -> p g c", c=KGRP * MW)
                g1v = g1.rearrange("p (g c) -> p g c", c=KGRP * SW)
                n_grp = q // KGRP
                for g in range(n_grp):
                    first = t == 0 and g < 2
                    last = t == n_tiles - 1 and g >= n_grp - 2
                    pst = psA if g % 2 == 0 else psB
                    nc.tensor.matmul(
                        pst[:], g1v[:, g, :], ssv[:, g, :],
                        start=first, stop=last,
                    )

            nc.vector.tensor_copy(stage[:, (2 * b) * FM : (2 * b + 1) * FM], psA[:])
            nc.vector.tensor_copy(stage[:, (2 * b + 1) * FM : (2 * b + 2) * FM], psB[:])

        nc.sync.dma_start(out[:], stage[:])

    nc.compile()
    return nc


_NC_CACHE = {}


def _get_nc():
    if "nc" not in _NC_CACHE:
        _NC_CACHE["nc"] = build_nc()
    return _NC_CACHE["nc"]


def fixup(Pm):
    """[Bt,FD,FM] grouped outer products -> [Bt,9,2] complex C entries.

    The KGRP diagonal [SW,MW] blocks each hold partial sums over voxels:
    rows 0:18 are kr-scaled, rows 18:36 ki-scaled Gsr components
    (2*(3i+j)+ta); cols are Grf components (2*(3j'+k)+tb).  The complex
    combine happens here.
    """
    Bt = Pm.shape[0]
    P36 = np.zeros((Bt, SW, MW), Pm.dtype)
    for k in range(KGRP):
        P36 += Pm[:, SW * k : SW * k + SW, MW * k : MW * k + MW]
    # rows: Grf comps (b); cols: [kr-scaled | ki-scaled] Gsr comps (a)
    P18r = P36[:, :, 0:18].transpose(0, 2, 1)
    P18i = P36[:, :, 18:36].transpose(0, 2, 1)
    ii, kk = np.mgrid[0:3, 0:3]
    cr = np.zeros((Bt, 3, 3), np.float32)
    ci = np.zeros((Bt, 3, 3), np.float32)
    for j in range(3):
        ae = 6 * ii + 2 * j
        be = 6 * j + 2 * kk
        cr += P18r[:, ae, be] - P18r[:, ae + 1, be + 1]
        cr += -P18i[:, ae + 1, be] - P18i[:, ae, be + 1]
        ci += P18r[:, ae, be + 1] + P18r[:, ae + 1, be]
        ci += P18i[:, ae, be] - P18i[:, ae + 1, be + 1]
    return np.stack([cr.reshape(Bt, 9), ci.reshape(Bt, 9)], axis=-1)


def run(x0, x1, x2, trace=False):
    from concourse.bass_utils import run_bass_kernel_spmd

    x0 = np.ascontiguousarray(np.asarray(x0), dtype=np.float16)
    x1 = np.ascontiguousarray(np.asarray(x1), dtype=np.float16)
    x2 = np.ascontiguousarray(np.asarray(x2), dtype=np.float16)
    assert x0.shape == (B_FULL, N_FULL, 9, 2), x0.shape

    nc = _get_nc()
    in_maps = [
        {
            "x0": x0[i * B_PC : (i + 1) * B_PC],
            "x1": x1[i * B_PC : (i + 1) * B_PC],
            "x2": x2[i * B_PC : (i + 1) * B_PC],
        }
        for i in range(N_CORES)
    ]
    res = None
    for attempt in range(4):
        try:
            res = run_bass_kernel_spmd(
                nc, in_maps, core_ids=list(range(N_CORES)), trace=trace
            )
        except Exception:
            # transient NRT device errors (e.g. NRT_EXEC_UNIT_UNRECOVERABLE
            # from a prior wedged run) clear on retry
            if attempt == 3:
                raise
            continue
        Pm = np.concatenate(
            [
                res.results[i]["out"]
                .reshape(FD, B_PC, 2, FM)
                .sum(axis=2)
                .transpose(1, 0, 2)
                for i in range(N_CORES)
            ],
            axis=0,
        )
        # transient device glitches can also surface as silent NaN/Inf
        # output; the inputs are finite so any non-finite result is a bad
        # run — retry it
        if np.isfinite(Pm).all():
            return fixup(Pm), res
    return fixup(Pm), res


def kernel(x0, x1, x2):
    out, _ = run(x0, x1, x2, trace=False)
    return out


# revision 34
# speedup vs baseline: 1.1586x; 1.1586x over previous
"""Trainium2 Bass kernel for ComputeGsct.

Math (per batch b, reduced over N voxels):
    kai(n)   = 10*x2[n,0] - i * x2[n,1]/(OMEGA*EPS0)          (complex scalar)
    A_n      = kai(n) * Gsr_n                                  (complex 3x3)
    C_b      = sum_n A_n @ Grf_n                               (complex 3x3)
    out[b,m,:] = (Re C_b, Im C_b) flattened row-major.

Strategy (v8):
  - Batch-parallel sharding: 8 cores x 4 batches each, full N per core.
    Output is concatenated on host - no cross-core reduction needed.
  - Host converts x0/x1 to f16 before upload.  The on-chip pipeline
    rounds Gsr/Grf to f16 before the matmuls anyway (PE runs f16), so
    this halves HBM traffic (79.7 -> 41.9 MB/core) at identical
    numerics class; rel err stays ~4e-4 vs the 2e-2 gate.
  - The complex combine lives OFF-DEVICE: per 128-voxel chunk the PE
    accumulates TWO [18,18]-blocked products into separate PSUM tiles,
      ps_r += (kr*Gsr)^T Grf      ps_i += (ki*Gsr)^T Grf
    (diag-block trick, KGRP chunks per matmul).  All eight real-product
    sums needed for the complex 3x3 result are linear combinations of
    ps_r/ps_i entries - a tiny host-side fixup extracts them.
  - On-chip per tile: two tiny ACT muls (kr, ki -> f16) and two f16
    DVE broadcast-muls (kr*Gsr, ki*Gsr).  Grf feeds the PE directly.
  - DMA: x0 on the sync(SP) ring, x1 on the scalar(ACT) ring - whole-
    tensor transfers only (splitting one tensor across rings interleaves
    the queues' packets on the shared DMA engines and HALVES per-engine
    throughput - measured).  xk rides the otherwise-idle Pool
    (gpsimd software-DGE) queue so the big rings are pure streamers.
"""

import sys

import numpy as np

_TRN_REPO = "/opt/trn_rl_repo"
if _TRN_REPO not in sys.path:
    sys.path.insert(0, _TRN_REPO)

_PAI = 3.141592653589793
_C = 299792458.0
_OMEGA = 2.0 * _PAI * 2.4e9
_MU0 = 4.0 * _PAI * 1e-7
_EPSILON0 = 1.0 / (_C**2 * _MU0)
_KI_SCALE = -1.0 / (_OMEGA * _EPSILON0)

B_FULL, N_FULL = 32, 131072
N_CORES = 8
B_PC = B_FULL // N_CORES  # batches per core
P = 128  # SBUF partitions == matmul contraction size
KGRP = 4  # voxel-chunks fused per matmul (diag-block trick)
SW = 18  # stationary cols per chunk: Grf
MW = 36  # moving cols per chunk: [kr*Gsr | ki*Gsr]
FD = KGRP * SW  # psum partition dim (72)
FM = KGRP * MW  # psum free dim (144)


def build_nc(b_pc=B_PC, n=N_FULL, q=128, repeat=1):
    """Build the per-core Bass program (SPMD: same program, per-core data)."""
    from contextlib import ExitStack

    import concourse.bacc as bacc
    import concourse.mybir as mybir
    from concourse import tile
    from concourse.bass import ts

    f32 = mybir.dt.float32
    f16 = mybir.dt.float16
    nc = bacc.Bacc("TRN2", target_bir_lowering=False, debug=False)

    x0 = nc.dram_tensor("x0", [b_pc, n, 9, 2], f16, kind="ExternalInput")
    x1 = nc.dram_tensor("x1", [b_pc, n, 9, 2], f16, kind="ExternalInput")
    x2 = nc.dram_tensor("x2", [b_pc, n, 2], f16, kind="ExternalInput")
    out = nc.dram_tensor("out", [FD, b_pc * 2 * FM], f32, kind="ExternalOutput")

    tile_v = P * q  # voxels per tile iteration
    assert n % tile_v == 0
    n_tiles = n // tile_v

    with ExitStack() as ctx:
        tc = ctx.enter_context(tile.TileContext(nc))
        io_g0 = ctx.enter_context(tc.tile_pool(name="io_g0", bufs=5))
        io_g1 = ctx.enter_context(tc.tile_pool(name="io_g1", bufs=5))
        io_xk = ctx.enter_context(tc.tile_pool(name="io_xk", bufs=6))
        work = ctx.enter_context(tc.tile_pool(name="work", bufs=6))
        psum = ctx.enter_context(tc.tile_pool(name="psum", bufs=2, space="PSUM"))
        outp = ctx.enter_context(tc.tile_pool(name="outp", bufs=1))

        if repeat > 1:
            loop = ctx.enter_context(tc.For_i(0, repeat, 1))  # noqa: F841

        stage = outp.tile([FD, b_pc * 2 * FM], f32)

        DT = 2  # compute-tiles per big-ring DMA (doubles descriptor size)
        assert n_tiles % DT == 0

        for b in range(b_pc):
            psA = psum.tile([FD, FM], f32, tag="psA")
            psB = psum.tile([FD, FM], f32, tag="psB")
            g0d = g1d = xkd = None
            for t in range(n_tiles):
                # ---- loads: voxel v = t*tile_v + p*q + qq, contiguous per
                # partition.  g0/g1 load DT tiles per transfer so each
                # descriptor is DT*q*18*2 bytes (bigger descriptors lift
                # the per-ring rate); xk stays per-tile on the Pool queue.
                if t % DT == 0:
                    td = t // DT
                    xkd = io_xk.tile([P, DT * q * 2], f16, tag="xk")
                    nc.gpsimd.dma_start(
                        xkd[:],
                        x2[b, ts(td, DT * tile_v)].rearrange(
                            "(p qq) r -> p (qq r)", p=P
                        ),
                    )
                    g0s = x0[b, ts(td, DT * tile_v)].rearrange(
                        "(p qq) m r -> p (qq m r)", p=P
                    )
                    g1s = x1[b, ts(td, DT * tile_v)].rearrange(
                        "(p qq) m r -> p (qq m r)", p=P
                    )
                    g0d = io_g0.tile([P, DT * q * 18], f16, tag="g0")
                    g1d = io_g1.tile([P, DT * q * 18], f16, tag="g1")
                    ends = (b == 0 and td == 0) or (
                        b == b_pc - 1 and td == n_tiles // DT - 1
                    )
                    if ends:
                        # halve the first/last transfers: the pipeline
                        # primes (first compute waits only ~1.5us) and
                        # drains (last compute starts earlier) faster.
                        HW_ = q * 18
                        nc.sync.dma_start(g0d[:, 0:HW_], g0s[:, 0:HW_])
                        nc.sync.dma_start(g0d[:, HW_:], g0s[:, HW_:])
                        nc.scalar.dma_start(g1d[:, 0:HW_], g1s[:, 0:HW_])
                        nc.scalar.dma_start(g1d[:, HW_:], g1s[:, HW_:])
                    else:
                        nc.sync.dma_start(g0d[:], g0s)
                        nc.scalar.dma_start(g1d[:], g1s)
                h = t % DT
                g0 = g0d[:, h * q * 18 : (h + 1) * q * 18]
                g1 = g1d[:, h * q * 18 : (h + 1) * q * 18]
                xk = xkd[:, h * q * 2 : (h + 1) * q * 2]

                # ---- kai components in f16, duplicated pairwise (ACT):
                # krd[p, 2*v] = krd[p, 2*v+1] = 10*x2[v,0].  The pair
                # duplication lets the DVE muls read kai through an AP
                # whose LAST dim is stride-1 x 2 elements — the packed-
                # last-dim requirement of DVE's 2x_1p fast path (a plain
                # stride-0 broadcast runs at 1 elem/cycle/lane).
                xkv = xk.rearrange("p (qq r) -> p qq r", r=2)
                krd = work.tile([P, q * 2], f16, tag="krd")
                krdv = krd[:].rearrange("p (qq d) -> p qq d", d=2)
                nc.vector.tensor_scalar_mul(krdv[:, :, 0], xkv[:, :, 0], 10.0)
                nc.vector.tensor_scalar_mul(krdv[:, :, 1], xkv[:, :, 0], 10.0)
                kid = work.tile([P, q * 2], f16, tag="kid")
                kidv = kid[:].rearrange("p (qq d) -> p qq d", d=2)
                nc.vector.tensor_scalar_mul(kidv[:, :, 0], xkv[:, :, 1], _KI_SCALE)
                nc.vector.tensor_scalar_mul(kidv[:, :, 1], xkv[:, :, 1], _KI_SCALE)

                # ---- scaled stationary (DVE, f16 2x path): interleaved
                # [kr*Gsr | ki*Gsr] per voxel chunk, 36 cols each.
                ss = work.tile([P, q * 36], f16, tag="ss")
                ss5 = ss[:].rearrange("p (qq h m r) -> p qq h m r", h=2, m=9, r=2)
                g0v4 = g0.rearrange("p (qq m r) -> p qq m r", m=9, r=2)
                nc.vector.tensor_mul(
                    ss5[:, :, 0],
                    g0v4,
                    krdv[:].unsqueeze(2).broadcast_to((P, q, 9, 2)),
                )
                nc.vector.tensor_mul(
                    ss5[:, :, 1],
                    g0v4,
                    kidv[:].unsqueeze(2).broadcast_to((P, q, 9, 2)),
                )

                # ---- TensorE: per 4-chunk group, ONE [128,72]^T@[128,144]
                # matmul (stationary = Grf, moving = interleaved scaled
                # Gsr); the 4 diagonal [18,36] blocks hold the wanted
                # kr/ki-scaled products.  PE is sequencer-dispatch-bound
                # (~30ns/inst), so fewer+fatter matmuls win.
                ssv = ss[:].rearrange("p (g c) -> p g c", c=KGRP * MW)
                g1v = g1.rearrange("p (g c) -> p g c", c=KGRP * SW)
                n_grp = q // KGRP
                for g in range(n_grp):
                    first = t == 0 and g < 2
                    last = t == n_tiles - 1 and g >= n_grp - 2
                    pst = psA if g % 2 == 0 else psB
                    nc.tensor.matmul(
                        pst[:], g1v[:, g, :], ssv[:, g, :],
                        start=first, stop=last,
                    )

            nc.vector.tensor_copy(stage[:, (2 * b) * FM : (2 * b + 1) * FM], psA[:])
            nc.vector.tensor_copy(stage[:, (2 * b + 1) * FM : (2 * b + 2) * FM], psB[:])

        nc.sync.dma_start(out[:], stage[:])

    nc.compile()
    return nc


_NC_CACHE = {}


def _get_nc():
    if "nc" not in _NC_CACHE:
        _NC_CACHE["nc"] = build_nc()
    return _NC_CACHE["nc"]


def fixup(Pm):
    """[Bt,FD,FM] grouped outer products -> [Bt,9,2] complex C entries.

    The KGRP diagonal [SW,MW] blocks each hold partial sums over voxels:
    rows 0:18 are kr-scaled, rows 18:36 ki-scaled Gsr components
    (2*(3i+j)+ta); cols are Grf components (2*(3j'+k)+tb).  The complex
    combine happens here.
    """
    Bt = Pm.shape[0]
    P36 = np.zeros((Bt, SW, MW), Pm.dtype)
    for k in range(KGRP):
        P36 += Pm[:, SW * k : SW * k + SW, MW * k : MW * k + MW]
    # rows: Grf comps (b); cols: [kr-scaled | ki-scaled] Gsr comps (a)
    P18r = P36[:, :, 0:18].transpose(0, 2, 1)
    P18i = P36[:, :, 18:36].transpose(0, 2, 1)
    ii, kk = np.mgrid[0:3, 0:3]
    cr = np.zeros((Bt, 3, 3), np.float32)
    ci = np.zeros((Bt, 3, 3), np.float32)
    for j in range(3):
        ae = 6 * ii + 2 * j
        be = 6 * j + 2 * kk
        cr += P18r[:, ae, be] - P18r[:, ae + 1, be + 1]
        cr += -P18i[:, ae + 1, be] - P18i[:, ae, be + 1]
        ci += P18r[:, ae, be + 1] + P18r[:, ae + 1, be]
        ci += P18i[:, ae, be] - P18i[:, ae + 1, be + 1]
    return np.stack([cr.reshape(Bt, 9), ci.reshape(Bt, 9)], axis=-1)


def run(x0, x1, x2, trace=False):
    from concourse.bass_utils import run_bass_kernel_spmd

    x0 = np.ascontiguousarray(np.asarray(x0), dtype=np.float16)
    x1 = np.ascontiguousarray(np.asarray(x1), dtype=np.float16)
    x2 = np.ascontiguousarray(np.asarray(x2), dtype=np.float16)
    assert x0.shape == (B_FULL, N_FULL, 9, 2), x0.shape

    nc = _get_nc()
    in_maps = [
        {
            "x0": x0[i * B_PC : (i + 1) * B_PC],
            "x1": x1[i * B_PC : (i + 1) * B_PC],
            "x2": x2[i * B_PC : (i + 1) * B_PC],
        }
        for i in range(N_CORES)
    ]
    res = None
    for attempt in range(4):
        try:
            res = run_bass_kernel_spmd(
                nc, in_maps, core_ids=list(range(N_CORES)), trace=trace
            )
        except Exception:
            # transient NRT device errors (e.g. NRT_EXEC_UNIT_UNRECOVERABLE
            # from a prior wedged run) clear on retry
            if attempt == 3:
                raise
            continue
        Pm = np.concatenate(
            [
                res.results[i]["out"]
                .reshape(FD, B_PC, 2, FM)
                .sum(axis=2)
                .transpose(1, 0, 2)
                for i in range(N_CORES)
            ],
            axis=0,
        )
        # transient device glitches can also surface as silent NaN/Inf
        # output; the inputs are finite so any non-finite result is a bad
        # run — retry it
        if np.isfinite(Pm).all():
            return fixup(Pm), res
    return fixup(Pm), res


def kernel(x0, x1, x2):
    out, _ = run(x0, x1, x2, trace=False)
    return out
